# revision 12
# baseline (speedup 1.0000x reference)
"""GQA attention kernel for 8 TRN2 NeuronCores.

Sharding: data-parallel over batch (B=2) x tensor-parallel over heads (4-way).
Core i handles batch i//4 and head-shard i%4 (8 query heads = 2 KV groups).
Out-proj is row-sharded; the 4 partial [S,D] outputs per batch are summed on
the host (cheap unshard step), bo added once.

Device kernel (per core, bf16 matmuls, f32 PSUM):
  KT = Wk_sh.T @ kvT          [128, S]
  V  = kvT.T  @ Wv_sh         [S, 128] -> per-group V_aug [S, 64+1] (ones col)
  QT = Wq_sh.T @ xT           [512, S]
  Attention runs per head-PAIR (hc, hc+4): the hperm layout puts group-0
  heads at partitions 0-63 and group-1 heads at 64-127, so the two QK^T
  matmuls of a pair occupy disjoint PE row-groups and execute concurrently
  (row tiling).  Causal masking is folded into the QK PSUM accumulation as
  an identity-matmul adding a -30208 lower-triangle, so exp(scale*s) is
  exactly 0 on masked entries and no vector-engine mask op exists on the
  exp->PV critical path.  One exp per pair covers both heads [128, 1024].
  PV accumulates with a ones-row giving softmax sums in row 64; normalize
  via DVE reciprocal + gpsimd partition_broadcast + one multiply.
  out_partial = OT.T @ Wo_sh  [S, D] bf16.
"""

import numpy as np

B, S, D = 2, 2048, 2048
H, G, HD, GS = 32, 8, 64, 4
HPC = 8     # query heads per core
NPAIR = 4   # head pairs per core (hc, hc+4)
NCORES = 8
SCALE = 0.125  # 1/sqrt(64)

_CACHE = {}


def _build():
    import concourse.bass as bass
    import concourse.tile as tile
    from concourse import bacc, mybir

    f32 = mybir.dt.float32
    bf16 = mybir.dt.bfloat16
    AF = mybir.ActivationFunctionType
    ALU = mybir.AluOpType

    nc = bacc.Bacc("TRN2", target_bir_lowering=False, debug=False,
                   num_devices=NCORES)

    xT_d = nc.declare_dram_parameter("xT", [D, S], bf16, isOutput=False)
    kvT_d = nc.declare_dram_parameter("kvT", [D, S], bf16, isOutput=False)
    wq_d = nc.declare_dram_parameter("wq", [D, 512], bf16, isOutput=False)
    wkv_d = nc.declare_dram_parameter("wkv", [D, 256], bf16, isOutput=False)
    wo_d = nc.declare_dram_parameter("wo", [512, D], bf16, isOutput=False)
    bq_d = nc.declare_dram_parameter("bq", [128, 4], f32, isOutput=False)
    bk_d = nc.declare_dram_parameter("bk", [128, 1], f32, isOutput=False)
    bvt_d = nc.declare_dram_parameter("bvt", [128, 2 * 64], f32, isOutput=False)
    mneg_d = nc.declare_dram_parameter("mneg", [128, 128], bf16, isOutput=False)
    i128_d = nc.declare_dram_parameter("i128", [128, 128], bf16, isOutput=False)
    out_d = nc.declare_dram_parameter("out", [S, D], bf16, isOutput=True)

    with tile.TileContext(nc) as tc:
        with (
            tc.tile_pool(name="persist", bufs=1) as persist,
            tc.tile_pool(name="stream", bufs=4) as stream,
            tc.tile_pool(name="probs", bufs=4) as probs_pool,
            tc.tile_pool(name="small", bufs=4) as small,
            tc.tile_pool(name="bsbp", bufs=4) as bsbp,
            tc.tile_pool(name="osum", bufs=4) as osump,
            tc.tile_pool(name="osbp", bufs=4) as osbp,
            tc.tile_pool(name="ps_s", bufs=2, space="PSUM") as ps_s,
            tc.tile_pool(name="ps_o", bufs=2, space="PSUM") as ps_o,
            tc.tile_pool(name="ps_p", bufs=2, space="PSUM") as ps_p,
        ):
            # ---- resident tiles ----
            wq_sb = persist.tile([128, 16 * 512], bf16, tag="wq")   # chunk c at c*512
            wkv_sb = persist.tile([128, 16 * 256], bf16, tag="wkv")  # [wk 128 | wv 128]
            wo_sb = persist.tile([128, 4 * 2048], bf16, tag="wo")
            mneg_sb = persist.tile([128, 128], bf16, tag="mneg")
            i128_sb = persist.tile([128, 128], bf16, tag="i128")
            bq_sb = persist.tile([128, 4], f32, tag="bq")
            bk_sb = persist.tile([128, 1], f32, tag="bk")
            bvt_sb = persist.tile([128, 2 * 64], f32, tag="bvt")

            qt_sb = persist.tile([128, 4 * 2048], bf16, tag="qt")   # chunk hc at hc*2048
            kt_sb = persist.tile([128, S], bf16, tag="kt")
            vaug_sb = persist.tile([128, 2 * 16 * 65], bf16, tag="vaug")
            ot_sb = persist.tile([128, 4 * 2048], bf16, tag="ot")

            # ---- scalar-engine act-table warmup (exp set loads at t~0) ----
            dmy = small.tile([1, 16], f32, tag="dmy")
            nc.vector.memset(dmy[:], 0.0)
            dmy2 = small.tile([1, 16], f32, tag="dmy2")
            nc.scalar.activation(dmy2[:], dmy[:], AF.Exp)

            # ---- chain emitters ----
            def kv_dma_piece(st, i):
                if i == 0:
                    st["kvt"] = stream.tile([128, 16 * 512], bf16, tag="xs",
                                            name="kvt")
                kvt, tch, c4 = st["kvt"], st["tch"], 4 * i
                nc.sync.dma_start(
                    out=kvt[:, c4 * 512:(c4 + 4) * 512]
                    .rearrange("p (c j) -> p c j", j=512),
                    in_=kvT_d[c4 * 128:(c4 + 4) * 128,
                              tch * 512:(tch + 1) * 512]
                    .rearrange("(c p) j -> p c j", p=128))

            def kv_dma(st):
                for i in range(4):
                    kv_dma_piece(st, i)

            def kv_compute_thunks(tch, st):
                th = []

                def kmm(c):
                    if c == 0:
                        st["kps"] = ps_p.tile([128, 512], f32, tag="pp",
                                              name="kps")
                    nc.tensor.matmul(
                        st["kps"][:], lhsT=wkv_sb[:, c * 256:c * 256 + 128],
                        rhs=st["kvt"][:, c * 512:(c + 1) * 512],
                        start=(c == 0), stop=(c == 15))
                    if c == 15:
                        nc.vector.tensor_scalar(
                            kt_sb[:, tch * 512:(tch + 1) * 512], st["kps"][:],
                            bk_sb[:, 0:1], None, ALU.add)
                for c in range(16):
                    th.append(lambda c=c: kmm(c))

                def vmm(tt, c):
                    if c == 0:
                        st["vps"] = ps_p.tile([128, 128], f32, tag="pp",
                                              name="vps")
                    nc.tensor.matmul(
                        st["vps"][:],
                        lhsT=st["kvt"][:, c * 512 + tt * 128:
                                       c * 512 + (tt + 1) * 128],
                        rhs=wkv_sb[:, c * 256 + 128:(c + 1) * 256],
                        start=(c == 0), stop=(c == 15))
                    if c == 15:
                        tok = tch * 4 + tt
                        for gl in range(2):
                            base = gl * 1040 + tok * 65
                            nc.vector.tensor_tensor(
                                vaug_sb[:, base:base + 64],
                                st["vps"][:, gl * 64:(gl + 1) * 64],
                                bvt_sb[:, gl * 64:(gl + 1) * 64], ALU.add)
                            nc.vector.memset(
                                vaug_sb[:, base + 64:base + 65], 1.0)
                for tt in range(4):
                    for c0 in range(0, 16, 4):
                        def v4(tt=tt, c0=c0):
                            for c in range(c0, c0 + 4):
                                vmm(tt, c)
                        th.append(v4)
                return th

            def q_dma_piece(st, i):
                if i == 0:
                    st["xt"] = stream.tile([128, 16 * 512], bf16, tag="xs",
                                           name="xt")
                xt, qch, c4 = st["xt"], st["qch"], 4 * i
                nc.sync.dma_start(
                    out=xt[:, c4 * 512:(c4 + 4) * 512]
                    .rearrange("p (c j) -> p c j", j=512),
                    in_=xT_d[c4 * 128:(c4 + 4) * 128,
                             qch * 512:(qch + 1) * 512]
                    .rearrange("(c p) j -> p c j", p=128))

            def q_dma(st):
                for i in range(4):
                    q_dma_piece(st, i)

            def q_hc_thunks(qch, hc, st):
                th = []

                def qmm(c):
                    if c == 0:
                        st["qps"] = ps_p.tile([128, 512], f32, tag="pp",
                                              name="qps")
                    nc.tensor.matmul(
                        st["qps"][:],
                        lhsT=wq_sb[:, c * 512 + hc * 128:c * 512 + (hc + 1) * 128],
                        rhs=st["xt"][:, c * 512:(c + 1) * 512],
                        start=(c == 0), stop=(c == 15))
                    if c == 15:
                        nc.vector.tensor_scalar(
                            qt_sb[:, hc * 2048 + qch * 512:
                                  hc * 2048 + (qch + 1) * 512],
                            st["qps"][:], bq_sb[:, hc:hc + 1], None, ALU.add)
                for c in range(16):
                    th.append(lambda c=c: qmm(c))
                return th

            tail_mode = [False]

            def outproj_thunks(jqb):
                th = []
                state = {}

                def omm(qt_i, cc, c):
                    if c == 0:
                        state["outp"] = ps_p.tile([128, 512], f32, tag="pp",
                                                  name="outp")
                    nc.tensor.matmul(
                        state["outp"][:],
                        lhsT=ot_sb[:, c * 2048 + qt_i * 128:
                                   c * 2048 + (qt_i + 1) * 128],
                        rhs=wo_sb[:, c * 2048 + cc * 512:c * 2048 + (cc + 1) * 512],
                        start=(c == 0), stop=(c == 3))
                    if c == 3:
                        if cc == 0:
                            state["osb"] = osbp.tile([128, 2048], bf16,
                                                     tag="osb", name="osb")
                        if tail_mode[0] and cc % 2 == 0:
                            # Act engine is idle once the last exp retired;
                            # alternate with DVE to halve the eviction chain
                            nc.scalar.activation(
                                state["osb"][:, cc * 512:(cc + 1) * 512],
                                state["outp"][:], AF.Copy)
                        else:
                            nc.vector.tensor_copy(
                                state["osb"][:, cc * 512:(cc + 1) * 512],
                                state["outp"][:])
                        if cc == 3:
                            nc.gpsimd.dma_start(
                                out=out_d[qt_i * 128:(qt_i + 1) * 128, :],
                                in_=state["osb"][:])
                for qt_i in range(jqb * 4, jqb * 4 + 4):
                    for cc in range(4):
                        for c in range(4):
                            th.append(lambda q=qt_i, cc=cc, c=c: omm(q, cc, c))
                return th

            # ---- filler queue machinery ----
            fillers = []
            fpos = [0]

            def pop_filler(n=1):
                while n > 0 and fpos[0] < len(fillers):
                    fillers[fpos[0]]()
                    fpos[0] += 1
                    n -= 1

            def drain_fillers_through(idx):
                while fpos[0] <= idx:
                    fillers[fpos[0]]()
                    fpos[0] += 1

            # ---- attention for one head pair (hc, hc+4) ----
            def attention(hc, jq):
                nkc = 4 * jq + 4
                qA = hc * 2048 + jq * 512
                kcol = lambda k: (k * 128, (k + 1) * 128)
                opsA = ps_o.tile([65, 512], f32, tag="ops", name="opsA")
                opsB = ps_o.tile([65, 512], f32, tag="ops", name="opsB")
                sps_t = {}
                pt_t = {}

                def emit_qk(kci):
                    m = max(0, kci * 128 - jq * 512)
                    diag = kci >= 4 * jq
                    k0, k1 = kcol(kci)
                    sps = ps_s.tile([128, 1024], f32, tag="sps", name="sps")
                    if diag:
                        nc.tensor.matmul(
                            sps[:, m:m + 128], lhsT=i128_sb[:], rhs=mneg_sb[:],
                            start=True, stop=False, skip_group_check=True)
                        nc.tensor.matmul(
                            sps[:, 512 + m:512 + m + 128], lhsT=i128_sb[:],
                            rhs=mneg_sb[:],
                            start=True, stop=False, skip_group_check=True)
                    nc.tensor.matmul(
                        sps[:, m:512], lhsT=kt_sb[0:64, k0:k1],
                        rhs=qt_sb[0:64, qA + m:qA + 512],
                        start=(not diag), stop=True, skip_group_check=diag)
                    nc.tensor.matmul(
                        sps[:, 512 + m:1024], lhsT=kt_sb[64:128, k0:k1],
                        rhs=qt_sb[64:128, qA + m:qA + 512],
                        start=(not diag), stop=True, skip_group_check=diag)
                    sps_t[kci] = (sps, m)

                def emit_exp(kci):
                    sps, m = sps_t.pop(kci)
                    pt = probs_pool.tile([128, 1024], bf16, tag="pt", name="pt")
                    nc.scalar.activation(pt[:, m:1024], sps[:, m:1024],
                                         AF.Exp, scale=SCALE)
                    pt_t[kci] = (pt, m)

                def emit_pv(kci, b):
                    pt, m = pt_t[kci] if b == 0 else pt_t.pop(kci)
                    vbase = b * 1040 + kci * 65
                    ops = opsB if b else opsA
                    nc.tensor.matmul(
                        ops[:, m:512], lhsT=vaug_sb[:, vbase:vbase + 65],
                        rhs=pt[:, b * 512 + m:b * 512 + 512],
                        start=(kci == 0), stop=(kci == nkc - 1))

                emit_qk(0)
                if nkc > 1:
                    emit_qk(1)
                for kci in range(nkc):
                    emit_exp(kci)
                    pop_filler(1)
                    if kci >= 1:
                        emit_pv(kci - 1, 1)
                    pop_filler(1)
                    emit_pv(kci, 0)
                    if kci + 2 < nkc:
                        emit_qk(kci + 2)
                emit_pv(nkc - 1, 1)

                # evict the PV accumulators to SBUF right away (frees the
                # PSUM slots; the [65,512] copy costs the same 512 rows as a
                # sums-row copy), then normalize from SBUF.
                for b, ops in ((0, opsA), (1, opsB)):
                    osum = osump.tile([64, 512], f32, tag="osum", name="osum")
                    nc.vector.tensor_copy(osum[:], ops[0:64, :])
                    rss = small.tile([1, 512], f32, tag="rss", name="rss")
                    nc.vector.tensor_copy(rss[:], ops[64:65, :])
                    rs = small.tile([1, 512], f32, tag="rs", name="rs")
                    nc.vector.reciprocal_approx_fast(rs[:], rss[:])
                    bsb = bsbp.tile([64, 512], f32, tag="bsb", name="bsb")
                    nc.gpsimd.partition_broadcast(bsb[:], rs[:], channels=64)
                    rows = 64 * b
                    nc.vector.tensor_tensor(
                        ot_sb[rows:rows + 64, qA:qA + 512],
                        osum[:], bsb[:], ALU.mult)

            # ---- prologue ----
            # single need-ordered sync queue: wkv/kv0 interleaved, then
            # consts, then x0/wq interleaved, then kv1/x1/wo.  bq + out go
            # on the gpsimd queue.
            kv_st = [{"tch": t} for t in range(4)]
            q_st = [{"qch": q} for q in range(4)]
            for i in range(4):
                nc.sync.dma_start(
                    out=wkv_sb[:, i * 4 * 256:(i + 1) * 4 * 256]
                    .rearrange("p (c j) -> p c j", j=256),
                    in_=wkv_d[i * 4 * 128:(i + 1) * 4 * 128, :]
                    .rearrange("(c p) j -> p c j", p=128))
                kv_dma_piece(kv_st[0], i)
            nc.sync.dma_start(out=bk_sb[:], in_=bk_d[:, :])
            nc.sync.dma_start(out=bvt_sb[:], in_=bvt_d[:, :])
            nc.sync.dma_start(out=mneg_sb[:], in_=mneg_d[:, :])
            nc.sync.dma_start(out=i128_sb[:], in_=i128_d[:, :])
            nc.gpsimd.dma_start(out=bq_sb[:], in_=bq_d[:, :])
            for i in range(4):
                q_dma_piece(q_st[0], i)
                nc.sync.dma_start(
                    out=wq_sb[:, i * 4 * 512:(i + 1) * 4 * 512]
                    .rearrange("p (c j) -> p c j", j=512),
                    in_=wq_d[i * 4 * 128:(i + 1) * 4 * 128, :]
                    .rearrange("(c p) j -> p c j", p=128))
            # kv0 + q0(hc0) compute inline
            for t in kv_compute_thunks(0, kv_st[0]):
                t()
            # prefetch next stream chunks + wo behind them
            kv_dma(kv_st[1])
            q_dma(q_st[1])
            for c in range(4):
                nc.sync.dma_start(
                    out=wo_sb[:, c * 2048:(c + 1) * 2048],
                    in_=wo_d[c * 128:(c + 1) * 128, :])
            for hc in range(4):
                for t in q_hc_thunks(0, hc, q_st[0]):
                    t()

            # fillers: q{jq} before kv{jq} so the kt/qt bias evicts complete
            # under the later drain matmuls instead of at the pair boundary
            group_end = {}

            def add_group(name, th):
                fillers.extend(th)
                group_end[name] = len(fillers) - 1

            add_group("q1", sum(
                (q_hc_thunks(1, hc, q_st[1]) for hc in range(4)), []))
            add_group("kv1", kv_compute_thunks(1, kv_st[1]))
            add_group("q2", [lambda: q_dma(q_st[2])] + sum(
                (q_hc_thunks(2, hc, q_st[2]) for hc in range(4)), []))
            add_group("kv2", [lambda: kv_dma(kv_st[2])] +
                      kv_compute_thunks(2, kv_st[2]))
            add_group("q3", [lambda: q_dma(q_st[3])] + sum(
                (q_hc_thunks(3, hc, q_st[3]) for hc in range(4)), []))
            add_group("kv3", [lambda: kv_dma(kv_st[3])] +
                      kv_compute_thunks(3, kv_st[3]))

            for jq in range(4):
                if jq >= 1:
                    drain_fillers_through(group_end[f"kv{jq}"])
                for hc in range(NPAIR):
                    attention(hc, jq)
                    pop_filler(2)
                fillers.extend(outproj_thunks(jq))
                group_end[f"op{jq}"] = len(fillers) - 1
            tail_mode[0] = True
            pop_filler(len(fillers))
    nc.finalize()
    return nc


def _get_nc():
    if "nc" not in _CACHE:
        _CACHE["nc"] = _build()
    return _CACHE["nc"]


def kernel(**inputs):
    out, _ = _run(inputs, trace=False)
    return out


def _run(inputs, trace=False):
    import ml_dtypes
    from concourse.bass_utils import run_bass_kernel_spmd

    x = np.asarray(inputs["x"], np.float32)
    kv = np.asarray(inputs["kv"], np.float32)
    Wq = np.asarray(inputs["Wq"], np.float32)
    bq = np.asarray(inputs["bq"], np.float32)
    Wk = np.asarray(inputs["Wk"], np.float32)
    bk = np.asarray(inputs["bk"], np.float32)
    Wv = np.asarray(inputs["Wv"], np.float32)
    bv = np.asarray(inputs["bv"], np.float32)
    Wo = np.asarray(inputs["Wo"], np.float32)
    bo = np.asarray(inputs["bo"], np.float32)

    bf = ml_dtypes.bfloat16
    # causal mask addend for a diagonal 128x128 chunk: row p masks cols t<p
    MNEG = np.where(np.arange(128)[None, :] < np.arange(128)[:, None],
                    -30208.0, 0.0).astype(bf)
    I128 = np.eye(128, dtype=np.float32).astype(bf)

    # head-dim permutation: chunk c = [local head c | local head 4+c]
    # so each head's Q rows sit at the partition half of its KV group.
    hperm = np.concatenate(
        [np.concatenate([np.arange(c * 64, c * 64 + 64),
                         np.arange((4 + c) * 64, (4 + c) * 64 + 64)])
         for c in range(4)])  # [512] permutation of local head dims

    in_maps = []
    for core in range(NCORES):
        b, t = core // 4, core % 4
        bv_sh = bv[t * 128:(t + 1) * 128]
        bvt = np.broadcast_to(bv_sh[None, :], (128, 128)).astype(np.float32)
        wq_sh = Wq[:, t * 512:(t + 1) * 512][:, hperm]
        wo_sh = Wo[t * 512:(t + 1) * 512, :][hperm, :]
        bq_sh = bq[t * 512:(t + 1) * 512][hperm]
        wkv_sh = np.concatenate(
            [Wk[:, t * 128:(t + 1) * 128], Wv[:, t * 128:(t + 1) * 128]],
            axis=1)
        in_maps.append({
            "xT": np.ascontiguousarray(x[b].T).astype(bf),
            "kvT": np.ascontiguousarray(kv[b].T).astype(bf),
            "wq": wq_sh.astype(bf),
            "wkv": np.ascontiguousarray(wkv_sh).astype(bf),
            "wo": np.ascontiguousarray(wo_sh).astype(bf),
            "bq": np.ascontiguousarray(bq_sh.reshape(4, 128).T),
            "bk": bk[t * 128:(t + 1) * 128].reshape(128, 1).copy(),
            "bvt": np.ascontiguousarray(bvt),
            "mneg": MNEG,
            "i128": I128,
        })

    nc = _get_nc()
    res = run_bass_kernel_spmd(nc, in_maps, core_ids=list(range(NCORES)),
                               trace=trace)
    parts = [np.asarray(res.results[i]["out"], np.float32)
             for i in range(NCORES)]
    out = np.stack([parts[0] + parts[1] + parts[2] + parts[3],
                    parts[4] + parts[5] + parts[6] + parts[7]])
    out += bo[None, None, :]
    return out.astype(np.float32), res
